# revision 11
# baseline (speedup 1.0000x reference)
"""GCN VAE-encoder kernel for Trainium2, distributed across 8 NeuronCores.

Model (see reference): 3x GCNConv+ReLU -> mu/logvar GCN heads ->
reparameterize, plus per-graph max-pool -> 2-layer MLP.

Distribution strategy:
  * Nodes are partitioned by graph (batch is sorted): core c owns graphs
    [c*64, (c+1)*64) and their nodes. Per-graph max pool + MLP are fully
    local to a core.
  * Each GCN aggregation is computed gather-first: for each edge, gather the
    *input* features h[row] (one dma_gather row per edge), scale by the
    symmetric-normalization weight via a selection matrix built on-chip, and
    segment-sum via TensorE matmuls into PSUM, grouped by 128-column output
    blocks. The dense transform (@ W) is applied afterwards at segment level
    (matmul and segment-sum commute), which keys gather traffic to F_in.
  * Between layers, per-core results are AllGather'd into a replicated
    feature table that the next layer's gathers read. The table is laid out
    chunk-major so each AllGather chunk fills one contiguous <=32767-row
    region (dma_gather has int16 indices).

The Bass program is identical on all 8 cores (SPMD); all per-core
variability lives in input tensors (gather indices, selection columns,
valid-edge counts loaded into registers at runtime).
"""

import math
import numpy as np

import concourse.bacc as bacc
import concourse.bass as bass
import concourse.tile as tile
from concourse import mybir
from concourse.bass_utils import run_bass_kernel_spmd
from concourse.masks import make_identity

F32 = mybir.dt.float32
_B_OVERRIDE = None  # set by small-scale tests; real problem has B=512
_DEBUG_TAPS = False  # expose intermediate h layers as outputs
_TRACE = False       # capture NTFF profile; result stored in _LAST_RESULTS
_LAST_RESULTS = None
I16 = mybir.dt.int16
I32 = mybir.dt.int32

NC = 8          # cores
GBLK = 4        # blocks (of 128 output nodes) per gather-call group
NCHUNK = 4      # int16 table chunks == AllGather chunks


def _ceil_to(x, m):
    return ((x + m - 1) // m) * m


# --------------------------------------------------------------------------
# Host-side preprocessing: build per-core tensors + static program metadata.
# --------------------------------------------------------------------------

class Meta:
    pass


def _preprocess(x, edge_index, batch, eps):
    m = Meta()
    N = x.shape[0]
    B = _B_OVERRIDE if _B_OVERRIDE is not None else 512
    GPC = B // NC
    m.N, m.B, m.GPC = N, B, GPC
    m.F0 = x.shape[1]

    batch = np.asarray(batch)
    # node ranges per core (batch is sorted)
    bounds = np.searchsorted(batch, np.arange(NC + 1) * GPC, side="left")
    nstart, nend = bounds[:-1], bounds[1:]
    n_c = nend - nstart
    NBLK = _ceil_to(int(math.ceil(n_c.max() / 128)), NCHUNK)
    SHARD = NBLK * 128
    CS = SHARD // NCHUNK              # rows per AG chunk per core
    CHUNKROWS = NC * CS               # rows per table chunk
    assert CHUNKROWS <= 32767, f"table chunk {CHUNKROWS} exceeds int16 range"
    TROWS = NC * SHARD
    m.NBLK, m.SHARD, m.CS, m.CHUNKROWS, m.TROWS = NBLK, SHARD, CS, CHUNKROWS, TROWS
    m.nstart, m.n_c = nstart, n_c

    # global table position of (core, local row)
    def pos(core, r):
        j = r // CS
        return j * CHUNKROWS + core * CS + (r % CS)

    # ---- x relaid into table layout (replicated to all cores) ----
    x_shard = np.zeros((TROWS, m.F0), dtype=np.float32)
    for c in range(NC):
        r = np.arange(n_c[c])
        x_shard[pos(c, r)] = x[nstart[c]:nend[c]]

    # ---- edges with self loops, normalization ----
    row = np.asarray(edge_index[0]).astype(np.int64)
    col = np.asarray(edge_index[1]).astype(np.int64)
    loop = np.arange(N, dtype=np.int64)
    row_f = np.concatenate([row, loop])
    col_f = np.concatenate([col, loop])
    deg = np.bincount(col_f, minlength=N).astype(np.float64)
    dinv = np.where(deg > 0, deg ** -0.5, 0.0)
    norm = (dinv[row_f] * dinv[col_f]).astype(np.float32)

    own_dst = batch[col_f] // GPC                 # owner core of the edge
    own_src = batch[row_f] // GPC
    r_dst = col_f - nstart[own_dst]               # local dest index
    r_src = row_f - nstart[own_src]
    tpos_src = (r_src // CS) * CHUNKROWS + own_src * CS + (r_src % CS)
    chunk = tpos_src // CHUNKROWS
    lidx = (tpos_src % CHUNKROWS).astype(np.int64)
    blk = r_dst // 128
    col_local = (r_dst % 128).astype(np.float32)

    NGRP = NBLK // GBLK
    NCALLS = NGRP * NCHUNK
    m.NGRP, m.NCALLS = NGRP, NCALLS

    # per (core, call): arrays of (lidx, col_local, norm, blk)
    per_call = [[None] * NCALLS for _ in range(NC)]
    for c in range(NC):
        sel = own_dst == c
        eb, el, ecl, en, ech = blk[sel], lidx[sel], col_local[sel], norm[sel], chunk[sel]
        grp = eb // GBLK
        callid = grp * NCHUNK + ech
        order = np.lexsort((eb, callid))
        eb, el, ecl, en, callid = eb[order], el[order], ecl[order], en[order], callid[order]
        cuts = np.searchsorted(callid, np.arange(NCALLS + 1))
        for k in range(NCALLS):
            s, e = cuts[k], cuts[k + 1]
            per_call[c][k] = [el[s:e], ecl[s:e], en[s:e], eb[s:e]]

    # guarantee >= 1 edge per call per core, and >= 1 incidence per block
    blocks_seen = np.zeros(NBLK, dtype=bool)
    for c in range(NC):
        for k in range(NCALLS):
            bs = per_call[c][k][3]
            blocks_seen[np.unique(bs)] = True
    for c in range(NC):
        for k in range(NCALLS):
            el, ecl, en, eb = per_call[c][k]
            g = k // NCHUNK
            need = [g * GBLK] if len(el) == 0 else []
            if k % NCHUNK == 0:
                for bb in range(g * GBLK, (g + 1) * GBLK):
                    if not blocks_seen[bb]:
                        need.append(bb)
            if need:
                need = sorted(set(need))
                el = np.concatenate([el, np.zeros(len(need), np.int64)])
                ecl = np.concatenate([ecl, np.zeros(len(need), np.float32)])
                en = np.concatenate([en, np.zeros(len(need), np.float32)])
                eb = np.concatenate([eb, np.asarray(need, np.int64)])
                order = np.argsort(eb, kind="stable")
                per_call[c][k] = [el[order], ecl[order], en[order], eb[order]]

    # static slot counts + incidence structure (shared across cores)
    S = np.zeros(NCALLS, dtype=np.int64)
    counts = np.zeros((NC, NCALLS), dtype=np.int32)
    for k in range(NCALLS):
        mx = 1
        for c in range(NC):
            counts[c, k] = len(per_call[c][k][0])
            mx = max(mx, len(per_call[c][k][0]))
        S[k] = _ceil_to(mx, 128)
    slot_off = np.zeros(NCALLS + 1, dtype=np.int64)
    np.cumsum(S, out=slot_off[1:])
    TOT_SLOTS = int(slot_off[-1])
    m.S, m.slot_off, m.TOT_SLOTS = S, slot_off, TOT_SLOTS

    # incidences: per call, list of (tile, block); per core selection columns
    incid = []           # list over calls of list of (tile, block)
    for k in range(NCALLS):
        ntile = S[k] // 128
        pairs = set()
        for c in range(NC):
            eb = per_call[c][k][3]
            for t in range(int(ntile)):
                for bb in np.unique(eb[t * 128:(t + 1) * 128]):
                    pairs.add((t, int(bb)))
        incid.append(sorted(pairs))
    inc_off = np.zeros(NCALLS + 1, dtype=np.int64)
    for k in range(NCALLS):
        inc_off[k + 1] = inc_off[k] + len(incid[k])
    NINC = int(inc_off[-1])
    m.incid, m.inc_off, m.NINC = incid, inc_off, NINC

    # start/stop flags per (group, block): first/last incidence in emission
    # order (calls ch=0..3 of the group, incidences in list order)
    first_flag = [[False] * len(incid[k]) for k in range(NCALLS)]
    last_flag = [[False] * len(incid[k]) for k in range(NCALLS)]
    for g in range(NGRP):
        for bb in range(g * GBLK, (g + 1) * GBLK):
            occ = []
            for ch in range(NCHUNK):
                k = g * NCHUNK + ch
                for i, (t, b2) in enumerate(incid[k]):
                    if b2 == bb:
                        occ.append((k, i))
            assert occ, f"block {bb} has no incidence"
            first_flag[occ[0][0]][occ[0][1]] = True
            last_flag[occ[-1][0]][occ[-1][1]] = True
    m.first_flag, m.last_flag = first_flag, last_flag

    # ---- pack per-core idx16 / colsel / normw ----
    idx16 = np.full((NC, 16, TOT_SLOTS // 16), -1, dtype=np.int16)
    colsel = np.zeros((NC, 128, NINC), dtype=np.float32)
    normw = np.zeros((NC, 128, NINC), dtype=np.float32)
    for c in range(NC):
        for k in range(NCALLS):
            el, ecl, en, eb = per_call[c][k]
            n = len(el)
            base = slot_off[k]
            i = np.arange(n)
            idx16[c, i % 16, base // 16 + i // 16] = el.astype(np.int16)
            for j, (t, bb) in enumerate(incid[k]):
                ic = inc_off[k] + j
                lo, hi = t * 128, min((t + 1) * 128, n)
                if lo >= n:
                    continue
                sl = slice(lo, hi)
                mask = eb[sl] == bb
                p = np.arange(lo, hi)[mask] - t * 128
                colsel[c, p, ic] = ecl[sl][mask]
                normw[c, p, ic] = en[sl][mask]
    idx16 = np.tile(idx16, (1, 8, 1))            # replicate to 128 partitions
    m.counts = counts

    # ---- pool tables ----
    gsize = np.bincount(batch, minlength=B)
    PMAX = _ceil_to(int(gsize.max()), 128)
    GSTR = 4                                       # graphs per stripe/call
    SPG = GSTR * PMAX                              # idxs per pool gather call
    NPCALL = GPC // GSTR
    PSLOTS = GPC * PMAX
    m.PMAX, m.GSTR, m.SPG, m.NPCALL, m.PSLOTS = PMAX, GSTR, SPG, NPCALL, PSLOTS
    pool_idx = np.full((NC, 16, PSLOTS // 16), -1, dtype=np.int16)
    gb = np.searchsorted(batch, np.arange(B + 1))
    for c in range(NC):
        flat = np.full(PSLOTS, m.SHARD, dtype=np.int16)   # pad -> -inf row
        for gl in range(GPC):
            gg = c * GPC + gl
            sz = gb[gg + 1] - gb[gg]
            flat[gl * PMAX: gl * PMAX + sz] = np.arange(
                gb[gg] - nstart[c], gb[gg + 1] - nstart[c], dtype=np.int16)
        i = np.arange(PSLOTS)
        pool_idx[c, i % 16, i // 16] = flat
    pool_idx = np.tile(pool_idx, (1, 8, 1))

    # ---- eps in core layout ----
    eps_c = np.zeros((NC, SHARD, eps.shape[1]), dtype=np.float32)
    for c in range(NC):
        eps_c[c, :n_c[c]] = eps[nstart[c]:nend[c]]

    iota = np.tile(np.arange(128, dtype=np.float32)[None, :], (128, 1))

    per_core = []
    for c in range(NC):
        per_core.append({
            "x_shard": x_shard,
            "eps_c": eps_c[c],
            "idx16": idx16[c],
            "colsel": colsel[c],
            "normw": normw[c],
            "counts": counts[c][None, :],
            "pool_idx": pool_idx[c],
            "iota": iota,
        })
    return m, per_core


# --------------------------------------------------------------------------
# Program builder (identical across cores).
# --------------------------------------------------------------------------

def _build_program(m, Ws, bs, fc1_w, fc1_b, fc2_w, fc2_b):
    """Ws/bs: lists of 4 weight matrices/biases; layer 4 is [Wmu|Wlv]."""
    F0 = m.F0
    FIN = [F0, Ws[0].shape[1], Ws[1].shape[1], Ws[2].shape[1]]
    FOUT = [W.shape[1] for W in Ws]
    FINAL = fc2_w.shape[1]
    SHARD, TROWS, NBLK = m.SHARD, m.TROWS, m.NBLK

    nc = bacc.Bacc("TRN2", target_bir_lowering=False, debug=False,
                   num_devices=NC, num_swdge_queues=4)

    # ---------------- DRAM tensors ----------------
    x_shard = nc.dram_tensor("x_shard", [TROWS, F0], F32, kind="ExternalInput")
    eps_in = nc.dram_tensor("eps_c", [SHARD, FIN[3]], F32, kind="ExternalInput")
    idx_in = nc.dram_tensor("idx16", [128, m.TOT_SLOTS // 16], I16,
                            kind="ExternalInput")
    colsel_in = nc.dram_tensor("colsel", [128, m.NINC], F32, kind="ExternalInput")
    normw_in = nc.dram_tensor("normw", [128, m.NINC], F32, kind="ExternalInput")
    counts_in = nc.dram_tensor("counts", [1, m.NCALLS], I32, kind="ExternalInput")
    pidx_in = nc.dram_tensor("pool_idx", [128, m.PSLOTS // 16], I16,
                             kind="ExternalInput")
    iota_in = nc.dram_tensor("iota", [128, 128], F32, kind="ExternalInput")

    wt_in, wb_in = [], []
    for li in range(4):
        wt_in.append(nc.dram_tensor(f"w{li}", [FIN[li], FOUT[li]], F32,
                                    kind="ExternalInput"))
        wb_in.append(nc.dram_tensor(f"wb{li}", [1, FOUT[li]], F32,
                                    kind="ExternalInput"))
    fc1w_in = nc.dram_tensor("fc1_w", [FIN[3], 1024], F32, kind="ExternalInput")
    fc1b_in = nc.dram_tensor("fc1_b", [128, 8], F32, kind="ExternalInput")
    fc2w_in = nc.dram_tensor("fc2_w", [1024, FINAL], F32, kind="ExternalInput")
    fc2b_in = nc.dram_tensor("fc2_b", [128, 1], F32, kind="ExternalInput")

    amvo_out = nc.dram_tensor("amvo", [SHARD, FIN[3]], F32, kind="ExternalOutput")
    mu_out = nc.dram_tensor("mu", [SHARD, FIN[3]], F32, kind="ExternalOutput")
    lv_out = nc.dram_tensor("lv", [SHARD, FIN[3]], F32, kind="ExternalOutput")
    pmvo_out = nc.dram_tensor("pmvoT", [FINAL, m.GPC], F32,
                              kind="ExternalOutput")

    agin = [None] * 4
    htab = [None] * 4
    hdbg = [None] * 4
    for li in range(3):
        extra = 1 if li == 2 else 0
        agin[li] = nc.dram_tensor(f"agin{li}", [SHARD + extra, FOUT[li]], F32,
                                  kind="Internal")
        htab[li] = nc.dram_tensor(f"htab{li}", [TROWS, FOUT[li]], F32,
                                  kind="Internal", addr_space="Shared")
        if _DEBUG_TAPS:
            hdbg[li] = nc.dram_tensor(f"hdbg{li}", [SHARD, FOUT[li]], F32,
                                      kind="ExternalOutput")
    tabdbg = None
    ptdbg = None
    if _DEBUG_TAPS:
        tabdbg = nc.dram_tensor("tabdbg", [TROWS, FOUT[1]], F32,
                                kind="ExternalOutput")
        ptdbg = nc.dram_tensor("ptdbg", [NBLK, 128, 256], F32,
                               kind="ExternalOutput")

    # gather source tables per layer: x for L1, htab[k-1] for L2..L4
    gtab = [x_shard, htab[0], htab[1], htab[2]]

    gregs = [nc.alloc_register(mybir.EngineType.Pool, f"gcnt{i}")
             for i in range(8)]

    MAXTILES = int(max(m.S)) // 128

    with tile.TileContext(nc) as tc:
        with (
            tc.tile_pool(name="const", bufs=1) as cpool,
            tc.tile_pool(name="gdst", bufs=3) as gpool,
            tc.tile_pool(name="sw", bufs=6) as swpool,
            tc.tile_pool(name="agg", bufs=5, space="PSUM") as aggpool,
            tc.tile_pool(name="tfp", bufs=2, space="PSUM") as tfpool,
            tc.tile_pool(name="ptp", bufs=1, space="PSUM") as ptpool,
            tc.tile_pool(name="work", bufs=3) as wpool,
            tc.tile_pool(name="epsw", bufs=3) as epool,
            tc.tile_pool(name="poolt", bufs=2) as plpool,
        ):
            # ---------- resident constants ----------
            idx_sb = cpool.tile([128, m.TOT_SLOTS // 16], I16)
            nc.sync.dma_start(idx_sb[:], idx_in[:])
            colsel_sb = cpool.tile([128, m.NINC], F32)
            nc.sync.dma_start(colsel_sb[:], colsel_in[:])
            normw_sb = cpool.tile([128, m.NINC], F32)
            nc.sync.dma_start(normw_sb[:], normw_in[:])
            counts_sb = cpool.tile([1, m.NCALLS], I32)
            nc.sync.dma_start(counts_sb[:], counts_in[:])
            pidx_sb = cpool.tile([128, m.PSLOTS // 16], I16)
            nc.sync.dma_start(pidx_sb[:], pidx_in[:])
            iota_sb = cpool.tile([128, 128], F32)
            nc.sync.dma_start(iota_sb[:], iota_in[:])
            ident = cpool.tile([128, 128], F32)
            make_identity(nc, ident[:])
            ones_sb = cpool.tile([1, 128], F32)
            nc.vector.memset(ones_sb[:], 1.0)

            wt_sb, wb_sb = [], []
            for li in range(4):
                kchunks = []
                for kc in range(0, FIN[li], 128):
                    kw = min(128, FIN[li] - kc)
                    t = cpool.tile([kw, FOUT[li]], F32, tag=f"w{li}_{kc}")
                    nc.sync.dma_start(t[:], wt_in[li][kc:kc + kw, :])
                    kchunks.append((t, kw))
                wt_sb.append(kchunks)
                t = cpool.tile([1, FOUT[li]], F32, tag=f"wb{li}")
                nc.sync.dma_start(t[:], wb_in[li][:])
                wb_sb.append(t)
            fc1w_sb = []
            for kc in range(0, FIN[3], 128):
                t = cpool.tile([128, 1024], F32, tag=f"fc1_{kc}")
                nc.sync.dma_start(t[:], fc1w_in[kc:kc + 128, :])
                fc1w_sb.append(t)
            fc1b_sb = cpool.tile([128, 8], F32)
            nc.sync.dma_start(fc1b_sb[:], fc1b_in[:])
            fc2w_sb = []
            for kc in range(0, 1024, 128):
                t = cpool.tile([128, FINAL], F32, tag=f"fc2_{kc}")
                nc.sync.dma_start(t[:], fc2w_in[kc:kc + 128, :])
                fc2w_sb.append(t)
            fc2b_sb = cpool.tile([128, 1], F32)
            nc.sync.dma_start(fc2b_sb[:], fc2b_in[:])
            x2T = [cpool.tile([128, m.GPC], F32, tag=f"x2T{h}", name=f"x2T{h}")
                   for h in range(2)]

            # -inf pad row for the pool gather source
            invrow = cpool.tile([1, FOUT[2]], F32)
            nc.vector.memset(invrow[:], -1e30)

            # warm up gather-dst slots: pad tails beyond the valid count are
            # never written by dma_gather, and stale SBUF could hold NaN
            # bit patterns that poison 0*NaN in the selection matmuls.
            for _ in range(3):
                wt_ = gpool.tile([128, MAXTILES, FIN[3]], F32, tag="gdst")
                nc.vector.memset(wt_[:].rearrange("p a b -> p (a b)"), 0.0)
            for _ in range(3):
                wt_ = gpool.tile([128, m.SPG // 128, FOUT[2]], F32, tag="pgd")
                nc.vector.memset(wt_[:].rearrange("p a b -> p (a b)"), 0.0)

            # ---------- GCN layers ----------
            def emit_layer(li):
                fin, fout = FIN[li], FOUT[li]
                nkc = (fin + 127) // 128
                table = gtab[li]
                for g in range(m.NGRP):
                    # PSUM accumulators for this group's blocks
                    pts = [aggpool.tile([128, nkc * 128], F32, tag="agg",
                                        name=f"agg{li}_{g}_{i}")
                           for i in range(GBLK)]
                    gts = []
                    for ch in range(NCHUNK):
                        k = g * NCHUNK + ch
                        ntile = int(m.S[k]) // 128
                        gt = gpool.tile([128, MAXTILES, fin], F32, tag="gdst")
                        gts.append(gt)
                        reg = gregs[(g * NCHUNK + ch) % len(gregs)]
                        nc.gpsimd.reg_load(reg, counts_sb[0:1, k:k + 1])
                        cs, ce = ch * m.CHUNKROWS, (ch + 1) * m.CHUNKROWS
                        nc.gpsimd.dma_gather(
                            gt[:, 0:ntile, :],
                            table[cs:ce, :],
                            idx_sb[:, int(m.slot_off[k]) // 16:
                                   int(m.slot_off[k] + m.S[k]) // 16],
                            int(m.S[k]), reg, fin,
                            queue_num=k % 4, single_packet=False,
                        )
                        for j, (t, bb) in enumerate(m.incid[k]):
                            ic = int(m.inc_off[k]) + j
                            sw = swpool.tile([128, 128], F32, tag="sw")
                            nc.vector.tensor_scalar(
                                sw[:], iota_sb[:],
                                colsel_sb[:, ic:ic + 1],
                                normw_sb[:, ic:ic + 1],
                                op0=mybir.AluOpType.is_equal,
                                op1=mybir.AluOpType.mult,
                            )
                            bl = bb - g * GBLK
                            st = m.first_flag[k][j]
                            sp = m.last_flag[k][j]
                            for kc in range(nkc):
                                kw = min(128, fin - kc * 128)
                                # start=True clears has_written for the WHOLE
                                # bank -- only the first matmul into this tile
                                # may set it, or it wipes the other chunk's
                                # accumulation state.
                                nc.tensor.matmul(
                                    pts[bl][0:kw, kc * 128:kc * 128 + 128],
                                    lhsT=gt[:, t, kc * 128:kc * 128 + kw],
                                    rhs=sw[:],
                                    start=st and kc == 0,
                                    stop=sp and kc == nkc - 1,
                                    skip_group_check=True,
                                )
                    # transform + epilogue per block
                    for bl in range(GBLK):
                        bb = g * GBLK + bl
                        pt_sb = wpool.tile([128, nkc * 128], F32, tag="ptsb")
                        for kc in range(nkc):
                            kw = min(128, fin - kc * 128)
                            nc.scalar.activation(
                                pt_sb[0:kw, kc * 128:kc * 128 + 128],
                                pts[bl][0:kw, kc * 128:kc * 128 + 128],
                                mybir.ActivationFunctionType.Copy,
                            )
                        if _DEBUG_TAPS and li == 2:
                            nc.sync.dma_start(
                                ptdbg[bb],
                                pt_sb[:, 0:256] if nkc * 128 >= 256
                                else pt_sb[:, 0:nkc * 128])
                        hps = tfpool.tile([128, fout], F32, tag="tf")
                        for kc in range(nkc):
                            kw = min(128, fin - kc * 128)
                            nc.tensor.matmul(
                                hps[:],
                                lhsT=pt_sb[0:kw, kc * 128:kc * 128 + 128],
                                rhs=wt_sb[li][kc][0][:],
                                start=(kc == 0), stop=False,
                            )
                        nc.tensor.matmul(
                            hps[:], lhsT=ones_sb[:], rhs=wb_sb[li][:],
                            start=False, stop=True,
                        )
                        if li < 3:
                            h_sb = wpool.tile([128, fout], F32, tag="hout")
                            nc.scalar.activation(
                                h_sb[:], hps[:],
                                mybir.ActivationFunctionType.Relu,
                            )
                            nc.sync.dma_start(
                                agin[li][bb * 128:(bb + 1) * 128, :], h_sb[:])
                            if _DEBUG_TAPS:
                                nc.sync.dma_start(
                                    hdbg[li][bb * 128:(bb + 1) * 128, :],
                                    h_sb[:])
                        else:
                            half = fout // 2
                            ml = wpool.tile([128, fout], F32, tag="hout")
                            nc.scalar.activation(
                                ml[:], hps[:],
                                mybir.ActivationFunctionType.Copy,
                            )
                            std = epool.tile([128, half], F32, tag="std")
                            nc.scalar.activation(
                                std[:], ml[:, half:],
                                mybir.ActivationFunctionType.Exp, scale=0.5,
                            )
                            ept = epool.tile([128, half], F32, tag="eps")
                            nc.sync.dma_start(
                                ept[:], eps_in[bb * 128:(bb + 1) * 128, :])
                            amv = epool.tile([128, half], F32, tag="amv")
                            nc.vector.tensor_tensor(
                                amv[:], ept[:], std[:], op=mybir.AluOpType.mult)
                            nc.vector.tensor_tensor(
                                amv[:], amv[:], ml[:, 0:half],
                                op=mybir.AluOpType.add)
                            rows = slice(bb * 128, (bb + 1) * 128)
                            nc.sync.dma_start(amvo_out[rows, :], amv[:])
                            nc.sync.dma_start(mu_out[rows, :], ml[:, 0:half])
                            nc.sync.dma_start(lv_out[rows, :], ml[:, half:])
                    # AllGather any finished chunk (layers 0..2)
                    if li < 3:
                        blocks_done = (g + 1) * GBLK
                        for j in range(NCHUNK):
                            boundary = (j + 1) * (NBLK // NCHUNK)
                            if blocks_done == boundary or (
                                    blocks_done > boundary and
                                    blocks_done - GBLK < boundary):
                                nc.gpsimd.collective_compute(
                                    "AllGather",
                                    mybir.AluOpType.bypass,
                                    replica_groups=[list(range(NC))],
                                    ins=[agin[li][j * m.CS:(j + 1) * m.CS, :]],
                                    outs=[htab[li][j * m.CHUNKROWS:
                                                   (j + 1) * m.CHUNKROWS, :]],
                                )

            emit_layer(0)
            emit_layer(1)
            emit_layer(2)
            if _DEBUG_TAPS:
                for bi in range(TROWS // 128):
                    tb = wpool.tile([128, FOUT[1]], F32, tag="tdbg", bufs=2)
                    nc.sync.dma_start(tb[:], htab[1][bi * 128:(bi + 1) * 128, :])
                    nc.sync.dma_start(tabdbg[bi * 128:(bi + 1) * 128, :], tb[:])

            # ---------- pool (overlaps with L4) ----------
            nc.sync.dma_start(agin[2][SHARD:SHARD + 1, :], invrow[:])

            def emit_pool():
                f3 = FOUT[2]
                for s in range(m.NPCALL):
                    pg = gpool.tile([128, m.SPG // 128, f3], F32, tag="pgd")
                    nc.gpsimd.dma_gather(
                        pg[:], agin[2][:, :],
                        pidx_sb[:, s * m.SPG // 16:(s + 1) * m.SPG // 16],
                        m.SPG, m.SPG, f3,
                        queue_num=s % 4, single_packet=False,
                    )
                    ptt = [plpool.tile([128, m.SPG], F32, tag=f"pT{h}",
                                       name=f"pT{h}_{s}")
                           for h in range(2)]
                    for t in range(m.SPG // 128):
                        for h in range(2):
                            tp = ptpool.tile([128, 128], F32, tag="ptp")
                            nc.tensor.transpose(
                                tp[:], pg[:, t, h * 128:(h + 1) * 128], ident[:])
                            nc.scalar.activation(
                                ptt[h][:, t * 128:(t + 1) * 128], tp[:],
                                mybir.ActivationFunctionType.Copy,
                            )
                    for gl in range(m.GSTR):
                        gg = s * m.GSTR + gl
                        for h in range(2):
                            nc.vector.tensor_reduce(
                                x2T[h][:, gg:gg + 1],
                                ptt[h][:, gl * m.PMAX:(gl + 1) * m.PMAX],
                                axis=mybir.AxisListType.X,
                                op=mybir.AluOpType.max,
                            )

            emit_pool()
            emit_layer(3)

            # ---------- FC head ----------
            y1 = []
            GPCW = m.GPC
            for mt in range(8):
                yp = tfpool.tile([128, GPCW], F32, tag="tf")
                for kc in range(len(fc1w_sb)):
                    nc.tensor.matmul(
                        yp[:], lhsT=fc1w_sb[kc][:, mt * 128:(mt + 1) * 128],
                        rhs=x2T[kc][:],
                        start=(kc == 0), stop=(kc == len(fc1w_sb) - 1),
                    )
                ys = wpool.tile([128, GPCW], F32, tag="y1", bufs=8)
                nc.scalar.activation(
                    ys[:], yp[:], mybir.ActivationFunctionType.Relu,
                    bias=fc1b_sb[:, mt:mt + 1],
                )
                y1.append(ys)
            pp = ptpool.tile([128, GPCW], F32, tag="ptp")
            for kc in range(8):
                nc.tensor.matmul(
                    pp[:], lhsT=fc2w_sb[kc][:], rhs=y1[kc][:],
                    start=(kc == 0), stop=(kc == 7),
                )
            pm = wpool.tile([128, GPCW], F32, tag="pmv", bufs=1)
            nc.vector.tensor_scalar_add(pm[:], pp[:], fc2b_sb[:, 0:1])
            nc.sync.dma_start(pmvo_out[:], pm[:])

    nc.compile()
    return nc


# --------------------------------------------------------------------------
# Entry point
# --------------------------------------------------------------------------

def kernel(x, edge_index, batch, eps, W1, b1, W2, b2, W3, b3, Wmu, bmu,
           Wlv, blv, fc1_w, fc1_b, fc2_w, fc2_b):
    x = np.asarray(x, dtype=np.float32)
    edge_index = np.asarray(edge_index)
    batch = np.asarray(batch)
    eps = np.asarray(eps, dtype=np.float32)

    meta, per_core = _preprocess(x, edge_index, batch, eps)

    Wcat = np.concatenate([np.asarray(Wmu), np.asarray(Wlv)], axis=1)
    bcat = np.concatenate([np.asarray(bmu), np.asarray(blv)])
    Ws = [np.asarray(W1, np.float32), np.asarray(W2, np.float32),
          np.asarray(W3, np.float32), Wcat.astype(np.float32)]
    bs = [np.asarray(b1, np.float32), np.asarray(b2, np.float32),
          np.asarray(b3, np.float32), bcat.astype(np.float32)]

    nc = _build_program(meta, Ws, bs, np.asarray(fc1_w), np.asarray(fc1_b),
                        np.asarray(fc2_w), np.asarray(fc2_b))

    fc1b_rs = np.asarray(fc1_b, np.float32).reshape(8, 128).T.copy()
    fc2b_rs = np.asarray(fc2_b, np.float32).reshape(128, 1)

    in_maps = []
    for c in range(NC):
        d = dict(per_core[c])
        d["w0"], d["wb0"] = Ws[0], bs[0][None, :]
        d["w1"], d["wb1"] = Ws[1], bs[1][None, :]
        d["w2"], d["wb2"] = Ws[2], bs[2][None, :]
        d["w3"], d["wb3"] = Ws[3], bs[3][None, :]
        d["fc1_w"] = np.asarray(fc1_w, np.float32)
        d["fc1_b"] = fc1b_rs
        d["fc2_w"] = np.asarray(fc2_w, np.float32)
        d["fc2_b"] = fc2b_rs
        in_maps.append(d)

    global _LAST_RESULTS
    res = run_bass_kernel_spmd(nc, in_maps, core_ids=list(range(NC)),
                               trace=_TRACE)
    _LAST_RESULTS = res

    N = meta.N
    half = Wcat.shape[1] // 2
    FINAL = np.asarray(fc2_w).shape[1]
    amvo = np.zeros((N, half), np.float32)
    mu = np.zeros((N, half), np.float32)
    lv = np.zeros((N, half), np.float32)
    pmvo = np.zeros((meta.B, FINAL), np.float32)
    for c in range(NC):
        r = res.results[c]
        s, n = meta.nstart[c], meta.n_c[c]
        amvo[s:s + n] = r["amvo"][:n]
        mu[s:s + n] = r["mu"][:n]
        lv[s:s + n] = r["lv"][:n]
        pmvo[c * meta.GPC:(c + 1) * meta.GPC] = r["pmvoT"].T
    return amvo, mu, lv, pmvo
